# revision 1
# baseline (speedup 1.0000x reference)
"""BinEmbedding kernel for Trainium2 (8 NeuronCores, data-parallel).

out[b, l, :] = emb_table[tok(x[b, l])]
  tok = 0 for NaN x, else clamp(searchsorted(bins, x, 'right') - 1, 0) + 1
      = [x >= -3e38] + sum_{j=1..255} [x >= bins[j]]   (exact fp32 is_ge; NaN
        compares false everywhere -> 0)

Per core: x slab [128, 1024] f32, element e = p*1024 + c at x_sb[p, c].
VectorE: 256 fused is_ge+add passes (bin thresholds baked as immediates),
cast to int16 tokens. Gather: SWDGE dma_gather of 256-B table rows, 1024
indices per call (128 calls) -- the call's indices are a 16-row band slice
of tok16, reshuffled within-partition by DVE into the HW's wrapped idx
order and replicated to partitions [0:32) (rx+tx Q7 cores) by one DMA per
col-block. Output lands so each dst partition holds 8 consecutive out rows
per call; stores use a split-partition AP (q outer, u inner).

Call (k, b, s): band k (x rows 16k..16k+16), col window C0 = b*256 + s*64.
  gather slot i = 128*s2 + 16*u + q  ->  dst[16u+q, s2]
  element e(q, u, s2) = (16k+q)*1024 + C0 + 8u + s2
  idx wrap: idx[q, 8*s2+u] = tok16[16k+q, C0 + 8u + s2]
"""

import sys

sys.path.insert(0, "/opt/trn_rl_repo")

import numpy as np

import concourse.bacc as bacc
import concourse.bass as bass
import concourse.mybir as mybir
from concourse.bass_utils import run_bass_kernel_spmd
from concourse.library_config import mlp

B, L = 16, 65536
NUM_BINS = 256
H = 64
P = 128
NCORES = 8

COLS = 1024
BLOCK_COLS = 256          # DVE compute block
SUB_COLS = 64             # cols per gather call
NI = 16 * SUB_COLS        # 1024 idxs per gather call (HW-safe limit)
NBUF = 16                 # dst ring depth (hides DMA latency)


def build_nc(bins: np.ndarray, cols: int = COLS):
    assert bins.shape == (NUM_BINS,) and bins.dtype == np.float32
    nblocks = cols // BLOCK_COLS
    subs_per_block = BLOCK_COLS // SUB_COLS        # 4
    calls_per_block = 8 * subs_per_block           # 32 (8 bands)
    ncalls = nblocks * calls_per_block

    thr = [-3.0e38] + [float(v) for v in bins[1:]]

    nc = bacc.Bacc("TRN2", target_bir_lowering=False, debug=False,
                   detect_race_conditions=False)
    x_d = nc.dram_tensor("x", [P, cols], mybir.dt.float32, kind="ExternalInput")
    emb_d = nc.dram_tensor(
        "emb", [NUM_BINS + 1, H], mybir.dt.float32, kind="ExternalInput"
    )
    out_d = nc.dram_tensor(
        "out", [P, cols * H], mybir.dt.float32, kind="ExternalOutput"
    )

    with (
        nc.sbuf_tensor("x_sb", [P, cols], mybir.dt.float32) as x_sb,
        nc.sbuf_tensor("acc", [P, cols], mybir.dt.float32) as acc,
        nc.sbuf_tensor("tok", [P, cols], mybir.dt.int16) as tok,
        nc.sbuf_tensor("strips", [P, cols], mybir.dt.int16) as strips,
        nc.sbuf_tensor("idxb", [P, (cols // SUB_COLS) * 8 * (NI // 16)], mybir.dt.int16) as idxb,
        nc.sbuf_tensor("dst", [P, NBUF, NI // P, H], mybir.dt.float32) as dst,
        nc.semaphore("sem_x") as sem_x,
        nc.semaphore("sem_strip") as sem_strip,
        nc.semaphore("sem_rep") as sem_rep,
        nc.semaphore("sem_v") as sem_v,
        nc.semaphore("sg0") as sg0,
        nc.semaphore("sg1") as sg1,
        nc.semaphore("sg2") as sg2,
        nc.semaphore("sg3") as sg3,
        nc.semaphore("ss0") as ss0,
        nc.semaphore("ss1") as ss1,
        nc.semaphore("ss2") as ss2,
        nc.semaphore("ss3") as ss3,
        nc.Block() as block,
    ):
        sem_gd = [sg0, sg1, sg2, sg3]
        sem_st = [ss0, ss1, ss2, ss3]

        # call index -> (b, k, s); processed in order b, then k-major, s inner
        def call_info(i):
            b, r = divmod(i, calls_per_block)
            k, s = divmod(r, subs_per_block)
            return b, k, s

        @block.vector
        def _(vector):
            # Intra-DVE RAW hazards are safe on HW (the per-op DRAIN is the
            # output-dependency barrier); only cross-engine edges get sems.
            vector.memset(strips[:, :], 0).then_inc(sem_strip, 1)
            vector.memset(idxb[:, :], 0).then_inc(sem_strip, 1)
            vector.memset(dst[:, :, :, :], 0)
            vector.wait_ge(sem_x, 16)
            for b in range(nblocks):
                lo, hi = b * BLOCK_COLS, (b + 1) * BLOCK_COLS
                xs = x_sb[:, lo:hi]
                ac = acc[:, lo:hi]
                vector.tensor_scalar(ac, xs, thr[0], None, mybir.AluOpType.is_ge)
                for j in range(1, NUM_BINS):
                    vector.scalar_tensor_tensor(
                        ac, xs, thr[j], ac,
                        mybir.AluOpType.is_ge, mybir.AluOpType.add,
                    )
                vector.tensor_copy(tok[:, lo:hi], ac)
                # strip shuffle: strips[16k+q, C0+8*s2+u] = tok[16k+q, C0+8u+s2]
                # one op per 32-partition band pair (legal start partitions)
                for m in range(4):
                    pr = slice(32 * m, 32 * m + 32)
                    o = strips[pr, lo:hi].rearrange(
                        "p (s a c) -> p s a c", a=8, c=8
                    )
                    i_ = tok[pr, lo:hi].rearrange(
                        "p (s c a) -> p s a c", c=8, a=8
                    )
                    vector.tensor_copy(o, i_).then_inc(sem_strip, 1)

        @block.scalar
        def _(scalar):
            # replicate strip bands into the wrapped idx tile, 2 copies
            # (partitions 0:16 rx and 16:32 tx), one DMA per col-block:
            # idxb[cp*16+q, b*2048 + (k*4+s)*64 + j] = strips[16k+q, b*256+s*64+j]
            for b in range(nblocks):
                scalar.wait_ge(sem_strip, 2 + 4 * (b + 1))
                if b >= 1:
                    # previous block's replication fully complete -> sem_rep
                    # milestones stay unambiguous (one block in flight)
                    scalar.wait_ge(sem_rep, 256 * b)
                for cp in range(2):
                    for k in range(8):
                        src_ap = strips[16 * k : 16 * k + 16,
                                        b * BLOCK_COLS : (b + 1) * BLOCK_COLS]
                        base = (b * calls_per_block + k * subs_per_block) * (NI // 16)
                        dst_ap = idxb[cp * 16 : cp * 16 + 16,
                                      base : base + BLOCK_COLS]
                        scalar.dma_start(dst_ap, src_ap).then_inc(sem_rep, 16)

        @block.gpsimd
        def _(gpsimd):
            gpsimd.load_library(mlp)
            for i in range(ncalls):
                b, k, s = call_info(i)
                gpsimd.wait_ge(sem_rep, 256 * (b + 1))
                if i >= NBUF:
                    j = i - NBUF
                    gpsimd.wait_ge(sem_st[j % 4], 16 * (j // 4 + 1))
                gpsimd.dma_gather(
                    dst[:, i % NBUF, :, :],
                    emb_d[:, :],
                    idxb[:, i * (NI // 16) : (i + 1) * (NI // 16)],
                    NI,
                    NI,
                    H,
                ).then_inc(sem_gd[i % 4], 16)

        @block.sync
        def _(sync):
            sync.dma_start(x_sb[:, :], x_d[:, :]).then_inc(sem_x, 16)
            for i in range(ncalls):
                b, k, s = call_info(i)
                c0 = b * BLOCK_COLS + s * SUB_COLS
                sync.wait_ge(sem_gd[i % 4], 16 * (i // 4 + 1))
                # dst[P = 16u+q, s2, h] -> out row (16k+q)*1024 + c0 + 8u + s2
                # SBUF side: natural partition order P (q fastest);
                # DRAM side: dims (u outer, q inner) to match.
                out_ap = bass.AP(
                    out_d,
                    (16 * k) * (cols * H) + c0 * H,
                    [
                        [8 * H, 8],          # u
                        [cols * H, 16],      # q (out rows)
                        [1, 8 * H],          # s2*H + h contiguous
                    ],
                )
                src_ap = dst[:, i % NBUF, :, :].rearrange("p a h -> p (a h)")
                sync.dma_start(out_ap, src_ap).then_inc(sem_st[i % 4], 16)

    nc.compile()
    return nc


_CACHE: dict = {}


def _get_nc(bins: np.ndarray):
    key = bins.tobytes()
    if key not in _CACHE:
        _CACHE[key] = build_nc(bins)
    return _CACHE[key]


def kernel(x: np.ndarray, bins: np.ndarray, emb_table: np.ndarray) -> np.ndarray:
    x = np.asarray(x, dtype=np.float32)
    bins = np.asarray(bins, dtype=np.float32)
    emb_table = np.asarray(emb_table, dtype=np.float32)
    assert x.shape == (B, L) and emb_table.shape == (NUM_BINS + 1, H)

    nc = _get_nc(bins)
    rows_per_core = B // NCORES
    in_maps = [
        {
            "x": x[i * rows_per_core : (i + 1) * rows_per_core].reshape(P, -1).copy(),
            "emb": emb_table,
        }
        for i in range(NCORES)
    ]
    res = run_bass_kernel_spmd(nc, in_maps, core_ids=list(range(NCORES)))
    outs = [
        res.results[i]["out"].reshape(rows_per_core, L, H) for i in range(NCORES)
    ]
    return np.concatenate(outs, axis=0)


if __name__ == "__main__":
    import concourse.bass_interp as bass_interp

    rng = np.random.default_rng(0)
    n = P * COLS
    bins = np.sort(rng.standard_normal(NUM_BINS).astype(np.float32) * 1.5)
    emb = rng.standard_normal((NUM_BINS + 1, H)).astype(np.float32)
    xs = rng.standard_normal(n).astype(np.float32)
    xs[rng.random(n) < 0.1] = np.nan

    nc = build_nc(bins)
    sim = bass_interp.CoreSim(nc, require_nnan=False, require_finite=False)
    sim.tensor("x")[:] = xs.reshape(P, COLS)
    sim.tensor("emb")[:] = emb
    sim.simulate()
    got = np.asarray(sim.tensor("out")).reshape(n, H)

    nans = np.isnan(xs)
    xc = np.where(nans, 0.0, xs)
    idx = np.maximum(np.searchsorted(bins, xc, side="right") - 1, 0)
    tok_ref = np.where(nans, 0, idx + 1)
    want = emb[tok_ref]
    err = np.abs(got - want).max()
    print("sim absmax err:", err)
    print("sim time estimate:", sim.time, "ns")
    assert err == 0.0, err
    print("SIM OK")



# revision 3
# speedup vs baseline: 2.1254x; 2.1254x over previous
"""BinEmbedding kernel v2 for Trainium2 (8 NeuronCores, data-parallel).

out[b, l, :] = emb_table[tok(x[b, l])],  tok = 0 for NaN else
clamp(searchsorted(bins, x, 'right') - 1, 0) + 1.

v1 used SWDGE dma_gather: ~12 ns/row of serial Q7 descriptor generation
= 1.6 ms GpSimd-bound.  v2 replaces the gather with an exact one-hot
matmul on the Tensor engine.  Per core, x slab [128, 1024] f32:

  phase 1 (DVE): 256 fused is_ge+add passes (thr = {-3e38, bins[1:]})
     -> count; tok = count + 288*isnan -> bf16 (all values exact ints)
  phase 2, pipelined over the 128 x-rows p:
   - SP:  2 KB SBUF->SBUF DMA stages tok row p to partition 0
   - PE:  ones-matmul broadcasts the staged row -> tokb psum [128, 1024]
   - Act: copies tokb psum -> SBUF bf16
   - DVE: is_equal vs per-partition token map -> 2 one-hot planes bf16
          (4x DVE mode; rows chunk0 = {288(nan), 1..127}, chunk1 =
          {128..255}; token 256 implied -> covered by the bias)
   - PE:  out[e, h] = sum_k O[k, e] * W[k, h]; W[k] = emb[token_k] - bias
          split into NSPLIT exact planes (fp16 x2 or bf16 x3, ulp-fixed
          host-side so the fp32 psum chain + bias reproduces emb[token]).
          8-way element interleave (group g = elements 8j+g) so each
          partition ends up with 8 consecutive output rows = 2 KB chunks.
   - Act: plain copy psum -> out SBUF f32
   - Pool: tensor_tensor add of the bias tile (gpsimd CAN do add)
   - SP:  2 KB-chunk HWDGE store to DRAM
"""

import sys

sys.path.insert(0, "/opt/trn_rl_repo")

import numpy as np

import concourse.bacc as bacc
import concourse.bass as bass
import concourse.mybir as mybir
from concourse.bass_utils import run_bass_kernel_spmd

B, L = 16, 65536
NUM_BINS = 256
H = 64
P = 128
NCORES = 8
COLS = 1024

NAN_TOK = 288.0          # bf16-exact, > any real token (1..256)

F16 = np.float16
BF = mybir.dt.np(mybir.dt.bfloat16)

# one-hot row -> token map (token 256 implied by all-zeros + bias)
ROW_TOKENS = [int(NAN_TOK)] + list(range(1, 128)) + list(range(128, 256))


def _split_fix(D, tgt, bias, dtypes):
    """Greedy exact split: d_i = cast(resid); last split ulp-searched so
    the device chain (((d1+d2)+...)+bias) lands on tgt in fp32 RN."""
    splits = []
    resid = D.copy()
    for dt in dtypes[:-1]:
        d = resid.astype(dt)
        splits.append(d)
        resid = (resid - d.astype(np.float32)).astype(np.float32)
    dlast = resid.astype(dtypes[-1])

    def chain(dl):
        s = np.zeros_like(D)
        for d in splits:
            s = (s + d.astype(np.float32)).astype(np.float32)
        s = (s + dl.astype(np.float32)).astype(np.float32)
        return (s + bias[None, :]).astype(np.float32)

    best, best_err = dlast, np.abs(chain(dlast) - tgt)
    di = dlast.view(np.int16)
    for step in (-2, -1, 1, 2):
        cand = (di + step).view(dtypes[-1])
        err = np.abs(chain(cand) - tgt)
        b = err < best_err
        best = np.where(b, cand, best)
        best_err = np.where(b, err, best_err)
    splits.append(best)
    rel = (best_err / np.maximum(np.abs(tgt), 1e-30)).max()
    return splits, rel


def build_tables(emb: np.ndarray):
    emb = emb.astype(np.float32)
    bias = emb[256].copy()
    tgt = np.stack([emb[0] if t == int(NAN_TOK) else emb[t] for t in ROW_TOKENS])
    D = (tgt - bias[None, :]).astype(np.float32)

    splits, rel = _split_fix(D, tgt, bias, [F16, F16])
    wdt, mmdt = F16, mybir.dt.float16
    if rel > 1e-3:
        splits, rel = _split_fix(D, tgt, bias, [BF, BF, BF])
        wdt, mmdt = BF, mybir.dt.bfloat16
        assert rel < 1e-3, f"weight chain fixup too lossy: rel={rel}"
    nsplit = len(splits)

    # w layout [128, nsplit*2*64]: block (c*nsplit+s) = split s of chunk c
    w = np.zeros((P, nsplit * 2 * H), dtype=wdt)
    for c in range(2):
        rows = slice(128 * c, 128 * (c + 1))
        for s, d in enumerate(splits):
            blk = c * nsplit + s
            w[:, blk * H:(blk + 1) * H] = d[rows].astype(wdt)

    kmap = np.zeros((P, 2), dtype=np.float32)
    kmap[:, 0] = ROW_TOKENS[:128]
    kmap[:, 1] = ROW_TOKENS[128:]
    biasrep = np.tile(bias[None, None, :], (P, 8, 1)).reshape(P, 8 * H)
    ones = np.ones((1, P), dtype=BF)
    tables = {"w": w, "kmap": kmap, "biasrep": biasrep.astype(np.float32),
              "ones": ones}
    return tables, nsplit, mmdt


def build_nc(bins: np.ndarray, nsplit: int, mmdt):
    assert bins.shape == (NUM_BINS,) and bins.dtype == np.float32
    thr = [-3.0e38] + [float(v) for v in bins[1:]]

    nc = bacc.Bacc("TRN2", target_bir_lowering=False, debug=False,
                   detect_race_conditions=False)
    x_d = nc.dram_tensor("x", [P, COLS], mybir.dt.float32, kind="ExternalInput")
    w_d = nc.dram_tensor("w", [P, nsplit * 2 * H], mmdt, kind="ExternalInput")
    kmap_d = nc.dram_tensor("kmap", [P, 2], mybir.dt.float32, kind="ExternalInput")
    brep_d = nc.dram_tensor("biasrep", [P, 8 * H], mybir.dt.float32, kind="ExternalInput")
    ones_d = nc.dram_tensor("ones", [1, P], mybir.dt.bfloat16, kind="ExternalInput")
    out_d = nc.dram_tensor("out", [P, COLS * H], mybir.dt.float32, kind="ExternalOutput")

    from contextlib import ExitStack
    with ExitStack() as ctx:
        x_sb = ctx.enter_context(nc.sbuf_tensor("x_sb", [P, COLS], mybir.dt.float32))
        cnt = ctx.enter_context(nc.sbuf_tensor("cnt", [P, COLS], mybir.dt.float32))
        nplane = ctx.enter_context(nc.sbuf_tensor("nplane", [P, COLS], mybir.dt.float32))
        tok16 = ctx.enter_context(nc.sbuf_tensor("tok16", [P, COLS], mybir.dt.bfloat16))
        tokstage = ctx.enter_context(nc.sbuf_tensor("tokstage", [1, 2, COLS], mybir.dt.bfloat16))
        w_sb = ctx.enter_context(nc.sbuf_tensor("w_sb", [P, nsplit * 2 * H], mmdt))
        kmap_sb = ctx.enter_context(nc.sbuf_tensor("kmap_sb", [P, 2], mybir.dt.float32))
        brep_sb = ctx.enter_context(nc.sbuf_tensor("brep_sb", [P, 8 * H], mybir.dt.float32))
        ones_sb = ctx.enter_context(nc.sbuf_tensor("ones_sb", [1, P], mybir.dt.bfloat16))
        tokb_sb = ctx.enter_context(nc.sbuf_tensor("tokb_sb", [P, 2, COLS], mybir.dt.bfloat16))
        oh_sb = ctx.enter_context(nc.sbuf_tensor("oh_sb", [P, 2, 2, COLS], mybir.dt.bfloat16))
        out_sb = ctx.enter_context(nc.sbuf_tensor("out_sb", [P, 2, 8 * H], mybir.dt.float32))
        tokb_ps = ctx.enter_context(nc.psum_tensor("tokb_ps", [P, 2, COLS], mybir.dt.float32))
        out_ps = ctx.enter_context(nc.psum_tensor("out_ps", [P, 2, 8 * H], mybir.dt.float32))
        s_x = ctx.enter_context(nc.semaphore("s_x"))
        s_c = ctx.enter_context(nc.semaphore("s_c"))
        s_tok = ctx.enter_context(nc.semaphore("s_tok"))
        s_ts = ctx.enter_context(nc.semaphore("s_ts"))
        s_bc = ctx.enter_context(nc.semaphore("s_bc"))
        s_tk = ctx.enter_context(nc.semaphore("s_tk"))
        s_eq = ctx.enter_context(nc.semaphore("s_eq"))
        s_mm = ctx.enter_context(nc.semaphore("s_mm"))
        s_cpA = ctx.enter_context(nc.semaphore("s_cpA"))
        s_bias = ctx.enter_context(nc.semaphore("s_bias"))
        s_st = ctx.enter_context(nc.semaphore("s_st"))
        block = ctx.enter_context(nc.Block())

        @block.sync
        def _(sync):
            sync.dma_start(x_sb[:, :], x_d[:, :]).then_inc(s_x, 16)
            sync.dma_start(w_sb[:, :], w_d[:, :]).then_inc(s_c, 16)
            sync.dma_start(kmap_sb[:, :], kmap_d[:, :]).then_inc(s_c, 16)
            sync.dma_start(brep_sb[:, :], brep_d[:, :]).then_inc(s_c, 16)
            sync.dma_start(ones_sb[:, :], ones_d[:, :]).then_inc(s_c, 16)
            sync.wait_ge(s_tok, 1)
            for p in range(P):
                # stage tok row p to partition 0 for the PE broadcast matmul
                if p >= 2:
                    sync.wait_ge(s_bc, p - 1)
                sync.dma_start(tokstage[0:1, p % 2, :],
                               tok16[p:p + 1, :]).then_inc(s_ts, 16)
                if p >= 2:
                    q = p - 2
                    sync.wait_ge(s_bias, q + 1)
                    out_ap = bass.AP(out_d, q * (COLS * H), [[8 * H, P], [1, 8 * H]])
                    sync.dma_start(out_ap, out_sb[:, q % 2, :]).then_inc(s_st, 16)
            for q in (P - 2, P - 1):
                sync.wait_ge(s_bias, q + 1)
                out_ap = bass.AP(out_d, q * (COLS * H), [[8 * H, P], [1, 8 * H]])
                sync.dma_start(out_ap, out_sb[:, q % 2, :]).then_inc(s_st, 16)

        @block.vector
        def _(vector):
            vector.wait_ge(s_x, 16)
            vector.tensor_tensor(nplane[:, :], x_sb[:, :], x_sb[:, :],
                                 mybir.AluOpType.not_equal)
            vector.tensor_scalar(cnt[:, :], x_sb[:, :], thr[0], None,
                                 mybir.AluOpType.is_ge)
            for j in range(1, NUM_BINS):
                vector.scalar_tensor_tensor(cnt[:, :], x_sb[:, :], thr[j],
                                            cnt[:, :], mybir.AluOpType.is_ge,
                                            mybir.AluOpType.add)
            # tok = nplane * NAN_TOK + count  (NaN: count=0 -> 288)
            vector.scalar_tensor_tensor(tok16[:, :], nplane[:, :], NAN_TOK,
                                        cnt[:, :], mybir.AluOpType.mult,
                                        mybir.AluOpType.add).then_inc(s_tok, 1)
            vector.wait_ge(s_c, 64)
            for p in range(P):
                r = p % 2
                vector.wait_ge(s_tk, p + 1)
                if p >= 2:
                    vector.wait_ge(s_mm, p - 1)
                for c in range(2):
                    vector.tensor_scalar(oh_sb[:, r, c, :], tokb_sb[:, r, :],
                                         kmap_sb[:, c:c + 1], None,
                                         mybir.AluOpType.is_equal)
                vector.engine_nop().then_inc(s_eq, 1)

        @block.scalar
        def _(scalar):
            for p in range(P):
                scalar.wait_ge(s_bc, p + 1)
                if p >= 2:
                    scalar.wait_ge(s_eq, p - 1)
                scalar.activation(tokb_sb[:, p % 2, :], tokb_ps[:, p % 2, :],
                                  mybir.ActivationFunctionType.Copy).then_inc(s_tk, 1)
                if p >= 1:
                    q = p - 1
                    scalar.wait_ge(s_mm, q + 1)
                    if q >= 2:
                        scalar.wait_ge(s_st, 16 * (q - 1))
                    scalar.activation(out_sb[:, q % 2, :], out_ps[:, q % 2, :],
                                      mybir.ActivationFunctionType.Copy
                                      ).then_inc(s_cpA, 1)
            q = P - 1
            scalar.wait_ge(s_mm, q + 1)
            scalar.wait_ge(s_st, 16 * (q - 1))
            scalar.activation(out_sb[:, q % 2, :], out_ps[:, q % 2, :],
                              mybir.ActivationFunctionType.Copy).then_inc(s_cpA, 1)

        @block.gpsimd
        def _(gpsimd):
            for p in range(P):
                gpsimd.wait_ge(s_cpA, p + 1)
                gpsimd.tensor_tensor(out_sb[:, p % 2, :], out_sb[:, p % 2, :],
                                     brep_sb[:, :],
                                     mybir.AluOpType.add).then_inc(s_bias, 1)

        @block.tensor
        def _(tensor):
            tensor.wait_ge(s_c, 64)

            def bcast(p):
                tensor.wait_ge(s_ts, 16 * (p + 1))
                if p >= 2:
                    tensor.wait_ge(s_tk, p - 1)
                # two N=512 matmuls: a matmul may not cross a psum bank
                tensor.matmul(tokb_ps[:, p % 2, 0:512], ones_sb[:, :],
                              tokstage[0:1, p % 2, 0:512], start=True, stop=True)
                tensor.matmul(tokb_ps[:, p % 2, 512:1024], ones_sb[:, :],
                              tokstage[0:1, p % 2, 512:1024], start=True,
                              stop=True).then_inc(s_bc, 1)

            bcast(0)
            bcast(1)
            for p in range(P):
                r = p % 2
                tensor.wait_ge(s_eq, p + 1)
                if p >= 2:
                    tensor.wait_ge(s_cpA, p - 1)
                for g in range(8):
                    out_ap = out_ps[:, r, g * H:(g + 1) * H]
                    for c in range(2):
                        lhsT = oh_sb[:, r, c, :].rearrange(
                            "p (j g) -> p g j", g=8)[:, g, :]
                        for s in range(nsplit):
                            mm = tensor.matmul(
                                out_ap, lhsT,
                                w_sb[:, (c * nsplit + s) * H:
                                     (c * nsplit + s + 1) * H],
                                start=(c == 0 and s == 0),
                                stop=(c == 1 and s == nsplit - 1),
                            )
                            if g == 7 and c == 1 and s == nsplit - 1:
                                mm.then_inc(s_mm, 1)
                if p + 2 < P:
                    bcast(p + 2)

    nc.compile()
    return nc


_CACHE: dict = {}


def _get_nc(bins: np.ndarray, nsplit: int = 2, mmdt=mybir.dt.float16):
    key = (bins.tobytes(), nsplit)
    if key not in _CACHE:
        _CACHE[key] = build_nc(bins, nsplit, mmdt)
    return _CACHE[key]


def kernel(x: np.ndarray, bins: np.ndarray, emb_table: np.ndarray) -> np.ndarray:
    x = np.asarray(x, dtype=np.float32)
    bins = np.asarray(bins, dtype=np.float32)
    emb_table = np.asarray(emb_table, dtype=np.float32)
    assert x.shape == (B, L) and emb_table.shape == (NUM_BINS + 1, H)

    tables, nsplit, mmdt = build_tables(emb_table)
    nc = _get_nc(bins, nsplit, mmdt)
    rows_per_core = B // NCORES
    in_maps = [
        {"x": x[i * rows_per_core:(i + 1) * rows_per_core].reshape(P, COLS).copy(),
         **tables}
        for i in range(NCORES)
    ]
    res = run_bass_kernel_spmd(nc, in_maps, core_ids=list(range(NCORES)))
    outs = [res.results[i]["out"].reshape(rows_per_core, L, H)
            for i in range(NCORES)]
    return np.concatenate(outs, axis=0)


if __name__ == "__main__":
    import concourse.bass_interp as bass_interp

    rng = np.random.default_rng(0)
    n = P * COLS
    bins = np.sort(rng.standard_normal(NUM_BINS).astype(np.float32) * 1.5)
    emb = rng.standard_normal((NUM_BINS + 1, H)).astype(np.float32)
    xs = rng.standard_normal(n).astype(np.float32)
    xs[rng.random(n) < 0.1] = np.nan

    tables, nsplit, mmdt = build_tables(emb)
    print("nsplit:", nsplit)
    nc = build_nc(bins, nsplit, mmdt)
    sim = bass_interp.CoreSim(nc, require_nnan=False, require_finite=False)
    sim.tensor("x")[:] = xs.reshape(P, COLS)
    for k, v in tables.items():
        sim.tensor(k)[:] = v
    sim.simulate()
    got = np.asarray(sim.tensor("out")).reshape(n, H)

    nans = np.isnan(xs)
    xc = np.where(nans, 0.0, xs)
    idx = np.maximum(np.searchsorted(bins, xc, side="right") - 1, 0)
    tok_ref = np.where(nans, 0, idx + 1)
    want = emb[tok_ref]
    abs_err = np.abs(got - want)
    rel = (abs_err / np.maximum(np.abs(want), 1e-30)).max()
    print("sim absmax err:", abs_err.max(), "rel:", rel)
    print("sim time estimate:", sim.time, "ns")
    assert rel < 1e-2, rel
    print("SIM OK")
